# revision 1
# baseline (speedup 1.0000x reference)
"""Trainium2 Bass kernel for nn_MLP_Interpolate.

Reference computation (out_size=512, H=W=128, so exact 4x nearest upsample):
  out[b, :, 4k+r, 4l+s] = relu(x[b,:,k,l] @ W1[:64] + c[r,s]) @ W2 + b2
  c[r,s] = rel_y(r)*W1[64] + rel_x(s)*W1[65] + b1,  rel(t) = (2t-3)/4

Strategy (8 cores, shard = (batch, H-half)):
  - F = W1c^T x computed on PE with a block-diagonal stationary so two
    64-channel pixel groups share one pass (128 partitions fully used).
  - 16 bias+relu variants split across ACT and DVE, written into an
    interleaved rhs tile ordered by *output* column (4l+s).
  - pred on PE with block-diag [128,6] W2 stationary -> PSUM rows are
    whole contiguous output rows, DMA'd straight to DRAM.
"""

import os

import numpy as np

import concourse.bass as bass
import concourse.bacc as bacc
import concourse.mybir as mybir
import concourse.tile as tile
from concourse.bass_utils import run_bass_kernel_spmd

# Problem constants (hardcoded per contract)
B, C, H, W = 4, 64, 128, 128
OUT = 512
NF = 64  # n_feat
N_CORES = 8
ROWS_PER_CORE = H // 2          # 64 input rows per core
T_TILES = ROWS_PER_CORE // 8    # 8 F-tiles, each covering 8 input rows
REL = np.array([-0.75, -0.25, 0.25, 0.75], dtype=np.float32)

_CACHE = {}


def _build_program():
    """Build + compile the SPMD Bass program once."""
    if "nc" in _CACHE:
        return _CACHE["nc"]

    fp32 = mybir.dt.float32
    # float32r: same bytes as fp32, PE streams 1 col/cycle vs 4 for fp32
    mm_dt = (mybir.dt.float32r if os.environ.get("MM_DTYPE") == "f32r"
             else fp32)
    nc = bacc.Bacc("TRN2", target_bir_lowering=False, debug=False,
                   num_devices=N_CORES)

    x_d = nc.dram_tensor("x", [C, ROWS_PER_CORE, W], mm_dt, kind="ExternalInput")
    w1_d = nc.dram_tensor("w1diag", [128, 128], mm_dt, kind="ExternalInput")
    w2_d = nc.dram_tensor("w2diag", [128, 6], mm_dt, kind="ExternalInput")
    crs_d = nc.dram_tensor("crsT", [128, 16], fp32, kind="ExternalInput")
    out_d = nc.dram_tensor("out", [3, 4 * ROWS_PER_CORE, OUT], fp32,
                           kind="ExternalOutput")

    NT = ROWS_PER_CORE // 16  # 4 F-tiles, each 16 input rows (8 per group)

    with tile.TileContext(nc) as tc:
        with (
            tc.tile_pool(name="consts", bufs=1) as consts,
            tc.tile_pool(name="xin", bufs=2) as xin,
            tc.tile_pool(name="hbuf", bufs=2) as hbuf,
            tc.tile_pool(name="stage", bufs=6) as stage,
            tc.tile_pool(name="fpsum", bufs=2, space=bass.MemorySpace.PSUM) as fpsum,
            tc.tile_pool(name="ppsum", bufs=2, space=bass.MemorySpace.PSUM) as ppsum,
        ):
            w1_sb = consts.tile([128, 128], mm_dt)
            w2_sb = consts.tile([128, 6], mm_dt)
            crs_sb = consts.tile([128, 16], fp32)
            nc.sync.dma_start(w1_sb[:], w1_d[:])
            nc.sync.dma_start(w2_sb[:], w2_d[:])
            nc.sync.dma_start(crs_sb[:], crs_d[:])

            x_tiles = []
            f_tiles = []

            def load_x(t):
                xt = xin.tile([128, 8, W], mm_dt, tag="xt")
                # group A: rows 16t..16t+8 -> partitions 0..63 (64 channels)
                nc.sync.dma_start(xt[0:64, :, :], x_d[:, 16 * t:16 * t + 8, :])
                # group B: rows 16t+8..16t+16 -> partitions 64..127
                nc.gpsimd.dma_start(xt[64:128, :, :],
                                    x_d[:, 16 * t + 8:16 * t + 16, :])
                x_tiles.append(xt)

            def feat_matmul(t):
                ft = fpsum.tile([128, 8, W], fp32, tag="ft")
                for half in range(2):
                    nc.tensor.matmul(ft[:, 4 * half:4 * half + 4, :],
                                     w1_sb[:],
                                     x_tiles[t][:, 4 * half:4 * half + 4, :],
                                     start=True, stop=True)
                f_tiles.append(ft)

            # 10 relu variants on ACT, 6 on DVE; copies 3 ACT / 5 DVE
            ACT_V = {0, 2, 4, 6, 8, 10, 12, 14, 15, 13}

            def tile_body(t):
                ft = f_tiles[t]
                for r in range(4):
                    # h tile [part, s, i, l]: relu writes contiguous runs;
                    # the output-column interleave (4l+s) happens in the
                    # matmul rhs read AP instead (strided reads are free on
                    # PE, strided writes are ~2.7x on ACT/DVE)
                    hr = hbuf.tile([128, 4, 8, W], mm_dt, tag="hr")
                    for s in range(4):
                        v = 4 * r + s
                        bias_ap = crs_sb[:, v:v + 1]
                        if v in ACT_V:
                            nc.scalar.activation(
                                hr[:, s, :, :], ft[:, :, :],
                                mybir.ActivationFunctionType.Relu,
                                bias=bias_ap)
                        else:
                            nc.vector.tensor_scalar(
                                hr[:, s, :, :], ft[:, :, :],
                                bias_ap, 0.0,
                                mybir.AluOpType.add, mybir.AluOpType.max)

                    def mm_rhs(i):
                        # [l, s] with s innermost -> streamed col n = 4l+s
                        return hr[:, :, i, :].rearrange("p s l -> p l s")

                    copy_idx = 0
                    for ihalf in range(2):
                        if mm_dt == fp32:
                            # i-quad at (partition 32*(ii//2), slot ii%2)
                            pt = ppsum.tile([38, 2, OUT], fp32, tag="pt")
                            for ii in range(4):
                                g, j = 32 * (ii // 2), ii % 2
                                nc.tensor.matmul(pt[g:g + 6, j, :], w2_sb[:],
                                                 mm_rhs(4 * ihalf + ii),
                                                 start=True, stop=True)
                            st = stage.tile([38, 2, OUT], fp32, tag="st")
                            if (r + ihalf) % 2 == 0:
                                nc.scalar.activation(
                                    st[:, :, :], pt[:, :, :],
                                    mybir.ActivationFunctionType.Copy)
                            else:
                                nc.vector.tensor_copy(st[:, :, :],
                                                      pt[:, :, :])
                            for q in range(2):
                                for grp in range(2):
                                    row = (64 * t + 16 * ihalf + 8 * q
                                           + 32 * grp + r)
                                    eng = (nc.gpsimd if (q + grp) % 2
                                           else nc.sync)
                                    eng.dma_start(
                                        out_d[:, row:row + 5:4, :],
                                        st[32 * q + 3 * grp:
                                           32 * q + 3 * grp + 3, :, :])
                        else:
                            # f32r: matmul dst base partition must be 0
                            st = stage.tile([6, 4, OUT], fp32, tag="st")
                            for jj in range(2):
                                pt = ppsum.tile([6, 2, OUT], fp32, tag="pt")
                                for j in range(2):
                                    i = 4 * ihalf + 2 * jj + j
                                    nc.tensor.matmul(pt[:, j, :], w2_sb[:],
                                                     mm_rhs(i),
                                                     start=True, stop=True)
                                # copies: 3 on ACT, 5 on DVE per r-loop pair
                                if copy_idx in (0, 3):
                                    nc.scalar.activation(
                                        st[:, 2 * jj:2 * jj + 2, :],
                                        pt[:, :, :],
                                        mybir.ActivationFunctionType.Copy)
                                else:
                                    nc.vector.tensor_copy(
                                        st[:, 2 * jj:2 * jj + 2, :],
                                        pt[:, :, :])
                                copy_idx += 1
                            for grp in range(2):
                                row = 64 * t + 16 * ihalf + 32 * grp + r
                                eng = nc.gpsimd if grp else nc.sync
                                eng.dma_start(
                                    out_d[:, row:row + 13:4, :],
                                    st[3 * grp:3 * grp + 3, :, :])

            # software pipeline: F(t+1) issued before preds(t) so ACT/DVE
            # for tile t+1 overlap PE pred work of tile t
            load_x(0)
            feat_matmul(0)
            for t in range(NT):
                if t + 1 < NT:
                    load_x(t + 1)
                    feat_matmul(t + 1)
                tile_body(t)

    nc.compile()
    _CACHE["nc"] = nc
    return nc


def _prep_inputs(x, W1, b1, W2, b2):
    x = np.ascontiguousarray(np.asarray(x, dtype=np.float32))
    W1 = np.asarray(W1, dtype=np.float32)
    b1 = np.asarray(b1, dtype=np.float32)
    W2 = np.asarray(W2, dtype=np.float32)

    w1c = W1[:NF]                      # [64, 64]
    w1diag = np.zeros((128, 128), dtype=np.float32)
    w1diag[0:64, 0:64] = w1c
    w1diag[64:128, 64:128] = w1c

    w2diag = np.zeros((128, 6), dtype=np.float32)
    w2diag[0:64, 0:3] = W2
    w2diag[64:128, 3:6] = W2

    # c[r,s] = rel[r]*W1[64] + rel[s]*W1[65] + b1 -> [16, 64]
    crs = (REL[:, None, None] * W1[NF][None, None, :]
           + REL[None, :, None] * W1[NF + 1][None, None, :]
           + b1[None, None, :]).reshape(16, NF)
    crsT = np.ascontiguousarray(
        np.concatenate([crs.T, crs.T], axis=0))  # [128, 16]

    in_maps = []
    for c in range(N_CORES):
        b, half = c // 2, c % 2
        xs = np.ascontiguousarray(
            x[b, :, half * ROWS_PER_CORE:(half + 1) * ROWS_PER_CORE, :])
        in_maps.append({"x": xs, "w1diag": w1diag, "w2diag": w2diag,
                        "crsT": crsT})
    return in_maps


def _gather(results, b2):
    full = np.empty((B, 3, OUT, OUT), dtype=np.float32)
    for c in range(N_CORES):
        b, half = c // 2, c % 2
        full[b, :, half * (OUT // 2):(half + 1) * (OUT // 2), :] = \
            results[c]["out"]
    b2 = np.asarray(b2, dtype=np.float32)
    if np.any(b2):
        full += b2.reshape(1, 3, 1, 1)
    return full


def run(trace=False, **inputs):
    nc = _build_program()
    in_maps = _prep_inputs(inputs["x"], inputs["W1"], inputs["b1"],
                           inputs["W2"], inputs["b2"])
    res = run_bass_kernel_spmd(nc, in_maps, list(range(N_CORES)), trace=trace)
    return _gather(res.results, inputs["b2"]), res


def kernel(**inputs):
    out, _ = run(trace=False, **inputs)
    return out



# revision 5
# speedup vs baseline: 1.6376x; 1.6376x over previous
"""Trainium2 Bass kernel for nn_MLP_Interpolate.

Reference computation (out_size=512, H=W=128 -> exact 4x nearest upsample):
  out[b, :, 4k+r, 4l+s] = relu(x[b,:,k,l] @ W1[:64] + c[r,s]) @ W2 + b2
  c[r,s] = rel_y(r)*W1[64] + rel_x(s)*W1[65] + b1,  rel(t) = (2t-3)/4

Strategy (8 cores, shard = (batch, H-half); all device math fp16 except
fp32 PSUM accumulation and fp32 output):
  - F = W1c^T x on PE with a 128x128 block-diagonal stationary so two
    64-channel pixel groups share each streamed column.
  - F copied PSUM->SBUF as fp16 (DVE), then all 16 bias+relu variants on
    DVE fp16 tensor_scalar (hits the 4X perf mode, ~0.35us per variant).
  - pred = h @ W2 on PE with a [128,6] block-diag stationary, packed 3
    matmuls per PSUM tile at column-tile positions 0/32/64, j-dim = r so
    each partition line holds 4 consecutive output rows.
  - ACT evacuates pred PSUM->SBUF staging; 16 large DMAs per tile-pair
    move [3, 2, 2048] chunks straight to DRAM.
"""

import numpy as np

import concourse.bass as bass
import concourse.bacc as bacc
import concourse.mybir as mybir
import concourse.tile as tile
from concourse.bass_utils import run_bass_kernel_spmd

# Problem constants (hardcoded per contract)
B, C, H, W = 4, 64, 128, 128
OUT = 512
NF = 64
N_CORES = 8
ROWS_PER_CORE = H // 2          # 64 input rows per core
NT = 4                          # tiles of 16 input rows (8 per group)
REL = np.array([-0.75, -0.25, 0.25, 0.75], dtype=np.float32)

_CACHE = {}


def _build_program():
    if "nc" in _CACHE:
        return _CACHE["nc"]

    fp32 = mybir.dt.float32
    fp16 = mybir.dt.float16
    nc = bacc.Bacc("TRN2", target_bir_lowering=False, debug=False,
                   num_devices=N_CORES)

    # x packed on host: [part = c + 64g, t, i*128 + l] where input row
    # rho = 16t + 8g + i
    x_d = nc.dram_tensor("x", [128, NT, 8 * W], fp16, kind="ExternalInput")
    w1_d = nc.dram_tensor("w1diag", [128, 128], fp16, kind="ExternalInput")
    w2_d = nc.dram_tensor("w2diag", [128, 32], fp16, kind="ExternalInput")
    crs_d = nc.dram_tensor("crsT", [128, 16], fp32, kind="ExternalInput")
    # out rows = 64t + 32g + 4i + r, flattened as [3, t, (row%64)*512 + l]
    out_d = nc.dram_tensor("out", [3, NT, 64 * OUT], fp32,
                           kind="ExternalOutput")

    with tile.TileContext(nc) as tc:
        with (
            tc.tile_pool(name="consts", bufs=1) as consts,
            tc.tile_pool(name="xbuf", bufs=1) as xbuf,
            tc.tile_pool(name="fbuf", bufs=2) as fbuf,
            tc.tile_pool(name="hbuf", bufs=2) as hbuf,
            tc.tile_pool(name="stbuf", bufs=2) as stbuf,
            tc.tile_pool(name="fpsum", bufs=2, space=bass.MemorySpace.PSUM) as fpsum,
            tc.tile_pool(name="ppsum", bufs=2, space=bass.MemorySpace.PSUM) as ppsum,
        ):
            w1_sb = consts.tile([128, 128], fp16)
            w2_sb = consts.tile([128, 32], fp16)
            crs_sb = consts.tile([128, 16], fp32)
            nc.sync.dma_start(w1_sb[:], w1_d[:])
            nc.sync.dma_start(w2_sb[:], w2_d[:])
            nc.sync.dma_start(crs_sb[:], crs_d[:])

            xall = xbuf.tile([128, NT, 8 * W], fp16)
            nc.sync.dma_start(xall[:, 0, :], x_d[:, 0, :])
            nc.sync.dma_start(xall[:, 1:NT, :], x_d[:, 1:NT, :])

            f_tiles = [None] * NT
            h_tiles = [None] * NT
            st_tiles = {}

            def feat_matmul(t, reps=1):
                ft = fpsum.tile([128, 8, W], fp32, tag="ft")
                for _ in range(reps):
                    for h in range(2):
                        nc.tensor.matmul(
                            ft[:, 4 * h:4 * h + 4, :], w1_sb[:],
                            xall[:, t, 512 * h:512 * h + 512],
                            start=True, stop=True)
                f_tiles[t] = ft

            def relus(t):
                # fb: fp16 copy of F so DVE tensor_scalar hits 4X mode
                fb = fbuf.tile([128, 8, W], fp16, tag="fb")
                nc.vector.tensor_copy(fb[:, :, :], f_tiles[t][:, :, :])
                hr = hbuf.tile([128, 16, 8, W], fp16, tag="hr")
                for v in range(16):
                    nc.vector.tensor_scalar(
                        hr[:, v, :, :], fb[:, :, :],
                        crs_sb[:, v:v + 1], 0.0,
                        mybir.AluOpType.add, mybir.AluOpType.max)
                h_tiles[t] = hr

            def pred(t):
                P = t // 2
                hr = h_tiles[t]
                if t % 2 == 0:
                    for m in range(3):
                        np_ = 70 if m < 2 else 38
                        stt = stbuf.tile([np_, 2, 4, OUT], fp32,
                                         tag=f"st{m}", name=f"st{m}")
                        st_tiles[(P, m)] = stt
                for rh in range(2):
                    for m in range(3):
                        nq = 3 if m < 2 else 2
                        pt = ppsum.tile([96, 2, OUT], fp32, tag="pt")
                        for q in range(nq):
                            i = 3 * q + m
                            for j in range(2):
                                r = 2 * rh + j
                                # rhs cols ordered (l-major, s inner) = 4l+s
                                rhs = hr[:, 4 * r:4 * r + 4, i, :] \
                                    .rearrange("p s l -> p l s")
                                nc.tensor.matmul(
                                    pt[32 * q:32 * q + 32, j, :],
                                    w2_sb[:], rhs, start=True, stop=True)
                        np_ = 70 if m < 2 else 38
                        nc.scalar.activation(
                            st_tiles[(P, m)][:, t % 2, 2 * rh:2 * rh + 2, :],
                            pt[0:np_, :, :],
                            mybir.ActivationFunctionType.Copy)

            def out_dmas(P, split):
                n = 0
                for m in range(3):
                    nq = 3 if m < 2 else 2
                    st = st_tiles[(P, m)]
                    for q in range(nq):
                        i = 3 * q + m
                        for g in range(2):
                            src = st[32 * q + 3 * g:32 * q + 3 * g + 3, :, :, :] \
                                .rearrange("p t r l -> p t (r l)")
                            fb0 = (32 * g + 4 * i) * OUT
                            dst = out_d[:, 2 * P:2 * P + 2, fb0:fb0 + 4 * OUT]
                            eng = nc.scalar if (split and n % 2) else nc.sync
                            eng.dma_start(dst, src)
                            n += 1

            feat_matmul(0)
            for t in range(NT):
                relus(t)
                if t + 1 < NT:
                    # extra reps keep the PE HAM-warm through the t=0
                    # relu window (PE would otherwise idle and throttle)
                    feat_matmul(t + 1, reps=6 if t == 0 else 1)
                pred(t)
                if t % 2 == 1:
                    out_dmas(t // 2, split=(t == NT - 1))

    nc.compile()
    _CACHE["nc"] = nc
    return nc


def _prep_inputs(x, W1, b1, W2, b2):
    x = np.asarray(x, dtype=np.float32)
    W1 = np.asarray(W1, dtype=np.float32)
    b1 = np.asarray(b1, dtype=np.float32)
    W2 = np.asarray(W2, dtype=np.float32)

    w1c = W1[:NF]                      # [64, 64]
    w1diag = np.zeros((128, 128), dtype=np.float16)
    w1diag[0:64, 0:64] = w1c
    w1diag[64:128, 64:128] = w1c

    w2diag = np.zeros((128, 32), dtype=np.float16)
    w2diag[0:64, 0:3] = W2
    w2diag[64:128, 3:6] = W2

    # c[v=4r+s, phi] = rel[r]*W1[64] + rel[s]*W1[65] + b1 -> [16, 64]
    crs = (REL[:, None, None] * W1[NF][None, None, :]
           + REL[None, :, None] * W1[NF + 1][None, None, :]
           + b1[None, None, :]).reshape(16, NF)
    crsT = np.ascontiguousarray(
        np.concatenate([crs.T, crs.T], axis=0)).astype(np.float32)  # [128,16]

    in_maps = []
    for c in range(N_CORES):
        b, half = c // 2, c % 2
        xs = x[b, :, half * ROWS_PER_CORE:(half + 1) * ROWS_PER_CORE, :]
        # [c, rho, l] -> [c, t, g, i, l] -> [g, c, t, i, l] -> [128, t, i*l]
        xp = np.ascontiguousarray(
            xs.reshape(NF, NT, 2, 8, W).transpose(2, 0, 1, 3, 4)
            .reshape(128, NT, 8 * W).astype(np.float16))
        in_maps.append({"x": xp, "w1diag": w1diag, "w2diag": w2diag,
                        "crsT": crsT})
    return in_maps


def _gather(results, b2):
    full = np.empty((B, 3, OUT, OUT), dtype=np.float32)
    for c in range(N_CORES):
        b, half = c // 2, c % 2
        o = results[c]["out"].reshape(3, 4 * ROWS_PER_CORE, OUT)
        full[b, :, half * (OUT // 2):(half + 1) * (OUT // 2), :] = o
    b2 = np.asarray(b2, dtype=np.float32)
    if np.any(b2):
        full += b2.reshape(1, 3, 1, 1)
    return full


def run(trace=False, **inputs):
    nc = _build_program()
    in_maps = _prep_inputs(inputs["x"], inputs["W1"], inputs["b1"],
                           inputs["W2"], inputs["b2"])
    res = run_bass_kernel_spmd(nc, in_maps, list(range(N_CORES)), trace=trace)
    return _gather(res.results, inputs["b2"]), res


def kernel(**inputs):
    out, _ = run(trace=False, **inputs)
    return out


# revision 8
# speedup vs baseline: 1.7816x; 1.0879x over previous
"""Trainium2 Bass kernel for nn_MLP_Interpolate.

Reference computation (out_size=512, H=W=128 -> exact 4x nearest upsample):
  out[b, :, 4k+r, 4l+s] = relu(x[b,:,k,l] @ W1[:64] + c[r,s]) @ W2 + b2
  c[r,s] = rel_y(r)*W1[64] + rel_x(s)*W1[65] + b1,  rel(t) = (2t-3)/4

Strategy (8 cores, shard = (batch, H-half); all device math fp16 except
fp32 PSUM accumulation and fp32 output):
  - F = W1c^T x on PE with a 128x128 block-diagonal stationary so two
    64-channel pixel groups share each streamed column.
  - F copied PSUM->SBUF as fp16 (DVE), then all 16 bias+relu variants on
    DVE fp16 tensor_scalar (hits the 4X perf mode, ~0.35us per variant).
  - pred = h @ W2 on PE with a [128,6] block-diag stationary, packed 3
    matmuls per PSUM tile at column-tile positions 0/32/64, j-dim = r so
    each partition line holds 4 consecutive output rows.
  - ACT evacuates pred PSUM->SBUF staging; 16 large DMAs per tile-pair
    move [3, 2, 2048] chunks straight to DRAM.
"""

import numpy as np

import concourse.bass as bass
import concourse.bacc as bacc
import concourse.mybir as mybir
import concourse.tile as tile
from concourse.bass_utils import run_bass_kernel_spmd

# Problem constants (hardcoded per contract)
B, C, H, W = 4, 64, 128, 128
OUT = 512
NF = 64
N_CORES = 8
ROWS_PER_CORE = H // 2          # 64 input rows per core
NT = 4                          # tiles of 16 input rows (8 per group)
REL = np.array([-0.75, -0.25, 0.25, 0.75], dtype=np.float32)

_CACHE = {}


def _build_program():
    if "nc" in _CACHE:
        return _CACHE["nc"]

    fp32 = mybir.dt.float32
    fp16 = mybir.dt.float16
    nc = bacc.Bacc("TRN2", target_bir_lowering=False, debug=False,
                   num_devices=N_CORES)

    # x packed on host: [part = c + 64g, t, i*128 + l] where input row
    # rho = 16t + 8g + i
    x_d = nc.dram_tensor("x", [128, NT, 8 * W], fp16, kind="ExternalInput")
    w1_d = nc.dram_tensor("w1diag", [128, 128], fp16, kind="ExternalInput")
    w2_d = nc.dram_tensor("w2diag", [128, 32], fp16, kind="ExternalInput")
    crs_d = nc.dram_tensor("crsT", [128, 16], fp32, kind="ExternalInput")
    # out rows = 64t + 32g + 4i + r, flattened as [3, t, (row%64)*512 + l]
    out_d = nc.dram_tensor("out", [3, NT, 64 * OUT], fp32,
                           kind="ExternalOutput")

    with tile.TileContext(nc) as tc:
        with (
            tc.tile_pool(name="consts", bufs=1) as consts,
            tc.tile_pool(name="xbuf", bufs=1) as xbuf,
            tc.tile_pool(name="fbuf", bufs=2) as fbuf,
            tc.tile_pool(name="hbuf", bufs=2) as hbuf,
            tc.tile_pool(name="stbuf", bufs=2) as stbuf,
            tc.tile_pool(name="fpsum", bufs=1, space=bass.MemorySpace.PSUM) as fpsum,
            tc.tile_pool(name="ppsum", bufs=3, space=bass.MemorySpace.PSUM) as ppsum,
        ):
            w1_sb = consts.tile([128, 128], fp16)
            w2_sb = consts.tile([128, 32], fp16)
            crs_sb = consts.tile([128, 16], fp32)
            xall = xbuf.tile([128, NT, 8 * W], fp16)
            nc.sync.dma_start(xall[:, 0, :], x_d[:, 0, :])
            nc.scalar.dma_start(w1_sb[:], w1_d[:])
            nc.scalar.dma_start(w2_sb[:], w2_d[:])
            nc.scalar.dma_start(crs_sb[:], crs_d[:])
            nc.sync.dma_start(xall[:, 1:NT, :], x_d[:, 1:NT, :])

            f_tiles = [None] * NT
            h_tiles = [None] * NT
            st_tiles = {}

            def feat_matmul(t, reps=1):
                ft = fpsum.tile([128, 8, W], fp32, tag="ft")
                for _ in range(reps):
                    for h in range(2):
                        nc.tensor.matmul(
                            ft[:, 4 * h:4 * h + 4, :], w1_sb[:],
                            xall[:, t, 512 * h:512 * h + 512],
                            start=True, stop=True)
                f_tiles[t] = ft

            def relus(t):
                # fb: fp16 copy of F so DVE tensor_scalar hits 4X mode
                fb = fbuf.tile([128, 8, W], fp16, tag="fb")
                nc.vector.tensor_copy(fb[:, :, :], f_tiles[t][:, :, :])
                hr = hbuf.tile([128, 16, 8, W], fp16, tag="hr")
                for v in range(16):
                    nc.vector.tensor_scalar(
                        hr[:, v, :, :], fb[:, :, :],
                        crs_sb[:, v:v + 1], 0.0,
                        mybir.AluOpType.add, mybir.AluOpType.max)
                h_tiles[t] = hr

            def dmas_for_m(P, m, engines, n0):
                # one DMA per (q, g): [3, 2, 2048] straight to DRAM
                nq = 3 if m < 2 else 2
                st = st_tiles[(P, m)]
                n = n0
                for q in range(nq):
                    i = 3 * q + m
                    for g in range(2):
                        src = st[32 * q + 3 * g:32 * q + 3 * g + 3, :, :, :] \
                            .rearrange("p t r l -> p t (r l)")
                        fb0 = (32 * g + 4 * i) * OUT
                        dst = out_d[:, 2 * P:2 * P + 2, fb0:fb0 + 4 * OUT]
                        engines[n % len(engines)].dma_start(dst, src)
                        n += 1
                return n

            def pred(t):
                P = t // 2
                hr = h_tiles[t]
                if t % 2 == 0:
                    for m in range(3):
                        np_ = 70 if m < 2 else 38
                        stt = stbuf.tile([np_, 2, 4, OUT], fp32,
                                         tag=f"st{m}", name=f"st{m}")
                        st_tiles[(P, m)] = stt
                ndma = 0
                for rh in range(2):
                    for m in range(3):
                        nq = 3 if m < 2 else 2
                        pt = ppsum.tile([96, 2, OUT], fp32, tag="pt")
                        for q in range(nq):
                            i = 3 * q + m
                            for j in range(2):
                                r = 2 * rh + j
                                # rhs cols ordered (l-major, s inner) = 4l+s
                                rhs = hr[:, 4 * r:4 * r + 4, i, :] \
                                    .rearrange("p s l -> p l s")
                                nc.tensor.matmul(
                                    pt[32 * q:32 * q + 32, j, :],
                                    w2_sb[:], rhs, start=True, stop=True)
                        np_ = 70 if m < 2 else 38
                        nc.scalar.activation(
                            st_tiles[(P, m)][:, t % 2, 2 * rh:2 * rh + 2, :],
                            pt[0:np_, :, :],
                            mybir.ActivationFunctionType.Copy)
                        if t % 2 == 1 and rh == 1:
                            # st pair complete for this m: stream it out now,
                            # spread across queues to avoid a serial tail
                            engines = ([nc.sync, nc.scalar, nc.gpsimd]
                                       if t == NT - 1
                                       else [nc.sync, nc.gpsimd])
                            ndma = dmas_for_m(P, m, engines, ndma)

            feat_matmul(0)
            for t in range(NT):
                relus(t)
                if t + 1 < NT:
                    # extra reps keep the PE HAM-warm through the t=0
                    # relu window (PE would otherwise idle and throttle)
                    feat_matmul(t + 1, reps=6 if t == 0 else 1)
                pred(t)

    nc.compile()
    _CACHE["nc"] = nc
    return nc


def _prep_inputs(x, W1, b1, W2, b2):
    x = np.asarray(x, dtype=np.float32)
    W1 = np.asarray(W1, dtype=np.float32)
    b1 = np.asarray(b1, dtype=np.float32)
    W2 = np.asarray(W2, dtype=np.float32)

    w1c = W1[:NF]                      # [64, 64]
    w1diag = np.zeros((128, 128), dtype=np.float16)
    w1diag[0:64, 0:64] = w1c
    w1diag[64:128, 64:128] = w1c

    w2diag = np.zeros((128, 32), dtype=np.float16)
    w2diag[0:64, 0:3] = W2
    w2diag[64:128, 3:6] = W2

    # c[v=4r+s, phi] = rel[r]*W1[64] + rel[s]*W1[65] + b1 -> [16, 64]
    crs = (REL[:, None, None] * W1[NF][None, None, :]
           + REL[None, :, None] * W1[NF + 1][None, None, :]
           + b1[None, None, :]).reshape(16, NF)
    crsT = np.ascontiguousarray(
        np.concatenate([crs.T, crs.T], axis=0)).astype(np.float32)  # [128,16]

    in_maps = []
    for c in range(N_CORES):
        b, half = c // 2, c % 2
        xs = x[b, :, half * ROWS_PER_CORE:(half + 1) * ROWS_PER_CORE, :]
        # [c, rho, l] -> [c, t, g, i, l] -> [g, c, t, i, l] -> [128, t, i*l]
        xp = np.ascontiguousarray(
            xs.reshape(NF, NT, 2, 8, W).transpose(2, 0, 1, 3, 4)
            .reshape(128, NT, 8 * W).astype(np.float16))
        in_maps.append({"x": xp, "w1diag": w1diag, "w2diag": w2diag,
                        "crsT": crsT})
    return in_maps


def _gather(results, b2):
    full = np.empty((B, 3, OUT, OUT), dtype=np.float32)
    for c in range(N_CORES):
        b, half = c // 2, c % 2
        o = results[c]["out"].reshape(3, 4 * ROWS_PER_CORE, OUT)
        full[b, :, half * (OUT // 2):(half + 1) * (OUT // 2), :] = o
    b2 = np.asarray(b2, dtype=np.float32)
    if np.any(b2):
        full += b2.reshape(1, 3, 1, 1)
    return full


def run(trace=False, **inputs):
    nc = _build_program()
    in_maps = _prep_inputs(inputs["x"], inputs["W1"], inputs["b1"],
                           inputs["W2"], inputs["b2"])
    res = run_bass_kernel_spmd(nc, in_maps, list(range(N_CORES)), trace=trace)
    return _gather(res.results, inputs["b2"]), res


def kernel(**inputs):
    out, _ = run(trace=False, **inputs)
    return out


# revision 10
# speedup vs baseline: 1.7826x; 1.0005x over previous
"""Trainium2 Bass kernel for nn_MLP_Interpolate.

Reference computation (out_size=512, H=W=128 -> exact 4x nearest upsample):
  out[b, :, 4k+r, 4l+s] = relu(x[b,:,k,l] @ W1[:64] + c[r,s]) @ W2 + b2
  c[r,s] = rel_y(r)*W1[64] + rel_x(s)*W1[65] + b1,  rel(t) = (2t-3)/4

Strategy (8 cores, shard = (batch, H-half); all device math fp16 except
fp32 PSUM accumulation and fp32 output):
  - F = W1c^T x on PE with a 128x128 block-diagonal stationary so two
    64-channel pixel groups share each streamed column.
  - F copied PSUM->SBUF as fp16 (DVE), then all 16 bias+relu variants on
    DVE fp16 tensor_scalar (hits the 4X perf mode, ~0.35us per variant).
  - pred = h @ W2 on PE with a [128,6] block-diag stationary, packed 3
    matmuls per PSUM tile at column-tile positions 0/32/64, j-dim = r so
    each partition line holds 4 consecutive output rows.
  - ACT evacuates pred PSUM->SBUF staging; 16 large DMAs per tile-pair
    move [3, 2, 2048] chunks straight to DRAM.
"""

import numpy as np

import concourse.bass as bass
import concourse.bacc as bacc
import concourse.mybir as mybir
import concourse.tile as tile
from concourse.bass_utils import run_bass_kernel_spmd

# Problem constants (hardcoded per contract)
B, C, H, W = 4, 64, 128, 128
OUT = 512
NF = 64
N_CORES = 8
ROWS_PER_CORE = H // 2          # 64 input rows per core
NT = 4                          # tiles of 16 input rows (8 per group)
REL = np.array([-0.75, -0.25, 0.25, 0.75], dtype=np.float32)

_CACHE = {}


def _build_program():
    if "nc" in _CACHE:
        return _CACHE["nc"]

    fp32 = mybir.dt.float32
    fp16 = mybir.dt.float16
    nc = bacc.Bacc("TRN2", target_bir_lowering=False, debug=False,
                   num_devices=N_CORES)

    # x packed on host: [part = c + 64g, t, i*128 + l] where input row
    # rho = 16t + 8g + i
    x_d = nc.dram_tensor("x", [128, NT, 8 * W], fp16, kind="ExternalInput")
    w1_d = nc.dram_tensor("w1diag", [128, 128], fp16, kind="ExternalInput")
    w2_d = nc.dram_tensor("w2diag", [128, 32], fp16, kind="ExternalInput")
    crs_d = nc.dram_tensor("crsT", [128, 16], fp32, kind="ExternalInput")
    # out rows = 64t + 32g + 4i + r, flattened as [3, t, (row%64)*512 + l]
    out_d = nc.dram_tensor("out", [3, NT, 64 * OUT], fp32,
                           kind="ExternalOutput")

    with tile.TileContext(nc) as tc:
        with (
            tc.tile_pool(name="consts", bufs=1) as consts,
            tc.tile_pool(name="xbuf", bufs=1) as xbuf,
            tc.tile_pool(name="fbuf", bufs=2) as fbuf,
            tc.tile_pool(name="hbuf", bufs=2) as hbuf,
            tc.tile_pool(name="stbuf", bufs=2) as stbuf,
            tc.tile_pool(name="fpsum", bufs=1, space=bass.MemorySpace.PSUM) as fpsum,
            tc.tile_pool(name="ppsum", bufs=3, space=bass.MemorySpace.PSUM) as ppsum,
        ):
            w1_sb = consts.tile([128, 128], fp16)
            w2_sb = consts.tile([128, 32], fp16)
            crs_sb = consts.tile([128, 16], fp32)
            xall = xbuf.tile([128, NT, 8 * W], fp16)
            nc.sync.dma_start(xall[:, 0, :], x_d[:, 0, :])
            nc.scalar.dma_start(w1_sb[:], w1_d[:])
            nc.scalar.dma_start(w2_sb[:], w2_d[:])
            nc.scalar.dma_start(crs_sb[:], crs_d[:])
            nc.sync.dma_start(xall[:, 1:NT, :], x_d[:, 1:NT, :])

            f_tiles = [None] * NT
            h_tiles = [None] * NT
            st_tiles = {}

            def feat_matmul(t, reps=1):
                ft = fpsum.tile([128, 8, W], fp32, tag="ft")
                for _ in range(reps):
                    for h in range(2):
                        nc.tensor.matmul(
                            ft[:, 4 * h:4 * h + 4, :], w1_sb[:],
                            xall[:, t, 512 * h:512 * h + 512],
                            start=True, stop=True)
                f_tiles[t] = ft

            def relus(t):
                # fb: fp16 copy of F so DVE tensor_scalar hits 4X mode
                fb = fbuf.tile([128, 8, W], fp16, tag="fb")
                nc.vector.tensor_copy(fb[:, :, :], f_tiles[t][:, :, :])
                hr = hbuf.tile([128, 16, 8, W], fp16, tag="hr")
                for v in range(16):
                    nc.vector.tensor_scalar(
                        hr[:, v, :, :], fb[:, :, :],
                        crs_sb[:, v:v + 1], 0.0,
                        mybir.AluOpType.add, mybir.AluOpType.max)
                h_tiles[t] = hr

            def dmas_for_m(P, m, engines, n0):
                # one DMA per (q, g): [3, 2, 2048] straight to DRAM
                nq = 3 if m < 2 else 2
                st = st_tiles[(P, m)]
                n = n0
                for q in range(nq):
                    i = 3 * q + m
                    for g in range(2):
                        src = st[32 * q + 3 * g:32 * q + 3 * g + 3, :, :, :] \
                            .rearrange("p t r l -> p t (r l)")
                        fb0 = (32 * g + 4 * i) * OUT
                        dst = out_d[:, 2 * P:2 * P + 2, fb0:fb0 + 4 * OUT]
                        engines[n % len(engines)].dma_start(dst, src)
                        n += 1
                return n

            def pred(t):
                P = t // 2
                hr = h_tiles[t]
                if t % 2 == 0:
                    for m in range(3):
                        np_ = 70 if m < 2 else 38
                        stt = stbuf.tile([np_, 2, 4, OUT], fp32,
                                         tag=f"st{m}", name=f"st{m}")
                        st_tiles[(P, m)] = stt
                ndma = 0
                # rh-major order lets early pred tiles start before the
                # rh=1 relus land; at t=3 all relus are long done, so go
                # m-major instead so each m's output DMAs start two tiles
                # earlier, shrinking the end-of-run DMA tail.
                if t == NT - 1:
                    order = [(m, rh) for m in range(3) for rh in range(2)]
                else:
                    order = [(m, rh) for rh in range(2) for m in range(3)]
                for m, rh in order:
                    nq = 3 if m < 2 else 2
                    pt = ppsum.tile([96, 2, OUT], fp32, tag="pt")
                    for q in range(nq):
                        i = 3 * q + m
                        for j in range(2):
                            r = 2 * rh + j
                            # rhs cols ordered (l-major, s inner) = 4l+s
                            rhs = hr[:, 4 * r:4 * r + 4, i, :] \
                                .rearrange("p s l -> p l s")
                            nc.tensor.matmul(
                                pt[32 * q:32 * q + 32, j, :],
                                w2_sb[:], rhs, start=True, stop=True)
                    np_ = 70 if m < 2 else 38
                    nc.scalar.activation(
                        st_tiles[(P, m)][:, t % 2, 2 * rh:2 * rh + 2, :],
                        pt[0:np_, :, :],
                        mybir.ActivationFunctionType.Copy)
                    if t % 2 == 1 and rh == 1:
                        # st pair complete for this m: stream it out now,
                        # spread across queues to avoid a serial tail
                        engines = ([nc.scalar, nc.sync, nc.gpsimd]
                                   if t == NT - 1
                                   else [nc.sync, nc.gpsimd])
                        ndma = dmas_for_m(P, m, engines, ndma)

            feat_matmul(0)
            for t in range(NT):
                relus(t)
                if t + 1 < NT:
                    # extra reps keep the PE HAM-warm through the relu
                    # windows (PE would otherwise micro-idle and throttle
                    # down to 1.2 GHz, slowing every subsequent matmul)
                    feat_matmul(t + 1, reps=6 if t == 0 else 3)
                pred(t)

    nc.compile()
    _CACHE["nc"] = nc
    return nc


def _prep_inputs(x, W1, b1, W2, b2):
    x = np.asarray(x, dtype=np.float32)
    W1 = np.asarray(W1, dtype=np.float32)
    b1 = np.asarray(b1, dtype=np.float32)
    W2 = np.asarray(W2, dtype=np.float32)

    w1c = W1[:NF]                      # [64, 64]
    w1diag = np.zeros((128, 128), dtype=np.float16)
    w1diag[0:64, 0:64] = w1c
    w1diag[64:128, 64:128] = w1c

    w2diag = np.zeros((128, 32), dtype=np.float16)
    w2diag[0:64, 0:3] = W2
    w2diag[64:128, 3:6] = W2

    # c[v=4r+s, phi] = rel[r]*W1[64] + rel[s]*W1[65] + b1 -> [16, 64]
    crs = (REL[:, None, None] * W1[NF][None, None, :]
           + REL[None, :, None] * W1[NF + 1][None, None, :]
           + b1[None, None, :]).reshape(16, NF)
    crsT = np.ascontiguousarray(
        np.concatenate([crs.T, crs.T], axis=0)).astype(np.float32)  # [128,16]

    in_maps = []
    for c in range(N_CORES):
        b, half = c // 2, c % 2
        xs = x[b, :, half * ROWS_PER_CORE:(half + 1) * ROWS_PER_CORE, :]
        # [c, rho, l] -> [c, t, g, i, l] -> [g, c, t, i, l] -> [128, t, i*l]
        xp = np.ascontiguousarray(
            xs.reshape(NF, NT, 2, 8, W).transpose(2, 0, 1, 3, 4)
            .reshape(128, NT, 8 * W).astype(np.float16))
        in_maps.append({"x": xp, "w1diag": w1diag, "w2diag": w2diag,
                        "crsT": crsT})
    return in_maps


def _gather(results, b2):
    full = np.empty((B, 3, OUT, OUT), dtype=np.float32)
    for c in range(N_CORES):
        b, half = c // 2, c % 2
        o = results[c]["out"].reshape(3, 4 * ROWS_PER_CORE, OUT)
        full[b, :, half * (OUT // 2):(half + 1) * (OUT // 2), :] = o
    b2 = np.asarray(b2, dtype=np.float32)
    if np.any(b2):
        full += b2.reshape(1, 3, 1, 1)
    return full


def run(trace=False, **inputs):
    nc = _build_program()
    in_maps = _prep_inputs(inputs["x"], inputs["W1"], inputs["b1"],
                           inputs["W2"], inputs["b2"])
    res = run_bass_kernel_spmd(nc, in_maps, list(range(N_CORES)), trace=trace)
    return _gather(res.results, inputs["b2"]), res


def kernel(**inputs):
    out, _ = run(trace=False, **inputs)
    return out


# revision 12
# speedup vs baseline: 2.0126x; 1.1291x over previous
"""Trainium2 Bass kernel for nn_MLP_Interpolate.

Reference computation (out_size=512, H=W=128 -> exact 4x nearest upsample):
  out[b, :, 4k+r, 4l+s] = relu(x[b,:,k,l] @ W1[:64] + c[r,s]) @ W2 + b2
  c[r,s] = rel_y(r)*W1[64] + rel_x(s)*W1[65] + b1,  rel(t) = (2t-3)/4

Strategy (8 cores, shard = (batch, H-half); all device math fp16 except
fp32 PSUM accumulation and fp32 output):
  - F = W1c^T x on PE with a 128x128 block-diagonal stationary so two
    64-channel pixel groups share each streamed column.
  - F copied PSUM->SBUF as fp16 (DVE cast), then all 16 bias+relu
    variants on DVE fp16 tensor_scalar (hits the 4X perf mode).
  - pred = h @ W2 on PE with a [128,32] zero-padded block-diag
    stationary, packed 3 matmuls per PSUM tile at column-tile positions
    0/32/64; j-dim = r so each partition line holds 4 consecutive
    output rows.  Stationary columns are ordered c-major (k = 2c+g) so
    one output DMA covers both pixel groups with a [3,2,2048] AP.
  - ACT evacuates pred PSUM->SBUF; 8 DMAs per tile stream [6,2048]
    chunks (48KB) straight to DRAM, rotated across queues.
"""

import numpy as np

import concourse.bass as bass
import concourse.bacc as bacc
import concourse.mybir as mybir
import concourse.tile as tile
from concourse.bass_utils import run_bass_kernel_spmd

# Problem constants (hardcoded per contract)
B, C, H, W = 4, 64, 128, 128
OUT = 512
NF = 64
N_CORES = 8
ROWS_PER_CORE = H // 2          # 64 input rows per core
NT = 4                          # tiles of 16 input rows (8 per group)
REL = np.array([-0.75, -0.25, 0.25, 0.75], dtype=np.float32)

_CACHE = {}


def _build_program():
    if "nc" in _CACHE:
        return _CACHE["nc"]

    fp32 = mybir.dt.float32
    fp16 = mybir.dt.float16
    nc = bacc.Bacc("TRN2", target_bir_lowering=False, debug=False,
                   num_devices=N_CORES)

    # x packed on host: [part = c + 64g, t, i*128 + l] where input row
    # rho = 16t + 8g + i
    x_d = nc.dram_tensor("x", [128, NT, 8 * W], fp16, kind="ExternalInput")
    w1_d = nc.dram_tensor("w1diag", [128, 128], fp16, kind="ExternalInput")
    w2_d = nc.dram_tensor("w2diag", [128, 32], fp16, kind="ExternalInput")
    crs_d = nc.dram_tensor("crsT", [128, 16], fp32, kind="ExternalInput")
    # out row = 64t + 32g + 4i + r, laid out [c, t, g, (4i+r)*512 + l]
    out_d = nc.dram_tensor("out", [3, NT, 2, 32 * OUT], fp32,
                           kind="ExternalOutput")

    with tile.TileContext(nc) as tc:
        with (
            tc.tile_pool(name="consts", bufs=1) as consts,
            tc.tile_pool(name="xbuf", bufs=1) as xbuf,
            tc.tile_pool(name="fbuf", bufs=2) as fbuf,
            tc.tile_pool(name="hbuf", bufs=2) as hbuf,
            tc.tile_pool(name="stbuf", bufs=2) as stbuf,
            tc.tile_pool(name="fpsum", bufs=1, space=bass.MemorySpace.PSUM) as fpsum,
            tc.tile_pool(name="ppsum", bufs=3, space=bass.MemorySpace.PSUM) as ppsum,
        ):
            w1_sb = consts.tile([128, 128], fp16)
            w2_sb = consts.tile([128, 32], fp16)
            crs_sb = consts.tile([128, 16], fp32)
            xall = xbuf.tile([128, NT, 8 * W], fp16)
            nc.sync.dma_start(xall[:, 0, :], x_d[:, 0, :])
            nc.scalar.dma_start(w1_sb[:], w1_d[:])
            nc.scalar.dma_start(w2_sb[:], w2_d[:])
            nc.scalar.dma_start(crs_sb[:], crs_d[:])
            nc.sync.dma_start(xall[:, 1:NT, :], x_d[:, 1:NT, :])

            f_tiles = [None] * NT
            h_tiles = [None] * NT
            st_tiles = {}

            def feat_matmul(t, reps=1):
                ft = fpsum.tile([128, 8, W], fp32, tag="ft")
                for _ in range(reps):
                    for h in range(2):
                        nc.tensor.matmul(
                            ft[:, 4 * h:4 * h + 4, :], w1_sb[:],
                            xall[:, t, 512 * h:512 * h + 512],
                            start=True, stop=True)
                f_tiles[t] = ft

            def relus(t):
                # fb: fp16 copy of F so DVE tensor_scalar hits 4X mode
                fb = fbuf.tile([128, 8, W], fp16, tag="fb")
                nc.vector.tensor_copy(fb[:, :, :], f_tiles[t][:, :, :])
                hr = hbuf.tile([128, 16, 8, W], fp16, tag="hr")
                for v in range(16):
                    nc.vector.tensor_scalar(
                        hr[:, v, :, :], fb[:, :, :],
                        crs_sb[:, v:v + 1], 0.0,
                        mybir.AluOpType.add, mybir.AluOpType.max)
                h_tiles[t] = hr

            def dmas_for_m(t, m, engines, n0):
                # one DMA per q: [6, 2048] covering both groups and all
                # four r rows of input row i = 3q + m
                nq = 3 if m < 2 else 2
                st = st_tiles[(t, m)]
                n = n0
                for q in range(nq):
                    i = 3 * q + m
                    src = st[32 * q:32 * q + 6, :, :] \
                        .rearrange("p r l -> p (r l)")
                    dst = out_d[:, t, :, 4 * i * OUT:(4 * i + 4) * OUT]
                    engines[n % len(engines)].dma_start(dst, src)
                    n += 1
                return n

            def pred(t):
                hr = h_tiles[t]
                for m in range(3):
                    np_ = 70 if m < 2 else 38
                    stt = stbuf.tile([np_, 4, OUT], fp32,
                                     tag=f"st{m}", name=f"st{m}")
                    st_tiles[(t, m)] = stt
                ndma = 0
                # m-major: each m's output DMAs can start as soon as its
                # two pred tiles are evacuated
                for m in range(3):
                    nq = 3 if m < 2 else 2
                    np_ = 70 if m < 2 else 38
                    for rh in range(2):
                        pt = ppsum.tile([96, 2, OUT], fp32, tag="pt")
                        for q in range(nq):
                            i = 3 * q + m
                            for j in range(2):
                                r = 2 * rh + j
                                # rhs cols ordered (l-major, s inner) = 4l+s
                                rhs = hr[:, 4 * r:4 * r + 4, i, :] \
                                    .rearrange("p s l -> p l s")
                                nc.tensor.matmul(
                                    pt[32 * q:32 * q + 32, j, :],
                                    w2_sb[:], rhs, start=True, stop=True)
                        nc.scalar.activation(
                            st_tiles[(t, m)][:, 2 * rh:2 * rh + 2, :],
                            pt[0:np_, :, :],
                            mybir.ActivationFunctionType.Copy)
                    engines = ([nc.scalar, nc.sync, nc.gpsimd]
                               if t == NT - 1 else [nc.sync, nc.gpsimd])
                    ndma = dmas_for_m(t, m, engines, ndma)

            feat_matmul(0)
            for t in range(NT):
                relus(t)
                if t + 1 < NT:
                    # extra reps keep the PE HAM-warm through the relu
                    # windows (PE would otherwise micro-idle and throttle
                    # down to 1.2 GHz, slowing every subsequent matmul)
                    feat_matmul(t + 1, reps=6 if t == 0 else 3)
                pred(t)

    nc.compile()
    _CACHE["nc"] = nc
    return nc


def _prep_inputs(x, W1, b1, W2, b2):
    x = np.asarray(x, dtype=np.float32)
    W1 = np.asarray(W1, dtype=np.float32)
    b1 = np.asarray(b1, dtype=np.float32)
    W2 = np.asarray(W2, dtype=np.float32)

    w1c = W1[:NF]                      # [64, 64]
    w1diag = np.zeros((128, 128), dtype=np.float16)
    w1diag[0:64, 0:64] = w1c
    w1diag[64:128, 64:128] = w1c

    # stationary columns k = 2c + g (c-major) so the output DMA's
    # partition iteration matches a [3, 2, 2048] DRAM AP
    w2diag = np.zeros((128, 32), dtype=np.float16)
    for g in range(2):
        for ch in range(3):
            w2diag[64 * g:64 * g + 64, 2 * ch + g] = W2[:, ch]

    # c[v=4r+s, phi] = rel[r]*W1[64] + rel[s]*W1[65] + b1 -> [16, 64]
    crs = (REL[:, None, None] * W1[NF][None, None, :]
           + REL[None, :, None] * W1[NF + 1][None, None, :]
           + b1[None, None, :]).reshape(16, NF)
    crsT = np.ascontiguousarray(
        np.concatenate([crs.T, crs.T], axis=0)).astype(np.float32)  # [128,16]

    in_maps = []
    for c in range(N_CORES):
        b, half = c // 2, c % 2
        xs = x[b, :, half * ROWS_PER_CORE:(half + 1) * ROWS_PER_CORE, :]
        # [c, rho, l] -> [c, t, g, i, l] -> [g, c, t, i, l] -> [128, t, i*l]
        xp = np.ascontiguousarray(
            xs.reshape(NF, NT, 2, 8, W).transpose(2, 0, 1, 3, 4)
            .reshape(128, NT, 8 * W).astype(np.float16))
        in_maps.append({"x": xp, "w1diag": w1diag, "w2diag": w2diag,
                        "crsT": crsT})
    return in_maps


def _gather(results, b2):
    full = np.empty((B, 3, OUT, OUT), dtype=np.float32)
    for c in range(N_CORES):
        b, half = c // 2, c % 2
        # [3, t, g, 32*512] -> rows ordered (t, g, 4i+r)
        o = results[c]["out"].reshape(3, 4 * ROWS_PER_CORE, OUT)
        full[b, :, half * (OUT // 2):(half + 1) * (OUT // 2), :] = o
    b2 = np.asarray(b2, dtype=np.float32)
    if np.any(b2):
        full += b2.reshape(1, 3, 1, 1)
    return full


def run(trace=False, **inputs):
    nc = _build_program()
    in_maps = _prep_inputs(inputs["x"], inputs["W1"], inputs["b1"],
                           inputs["W2"], inputs["b2"])
    res = run_bass_kernel_spmd(nc, in_maps, list(range(N_CORES)), trace=trace)
    return _gather(res.results, inputs["b2"]), res


def kernel(**inputs):
    out, _ = run(trace=False, **inputs)
    return out


# revision 14
# speedup vs baseline: 2.1883x; 1.0873x over previous
"""Trainium2 Bass kernel for nn_MLP_Interpolate.

Reference computation (out_size=512, H=W=128 -> exact 4x nearest upsample):
  out[b, :, 4k+r, 4l+s] = relu(x[b,:,k,l] @ W1[:64] + c[r,s]) @ W2 + b2
  c[r,s] = rel_y(r)*W1[64] + rel_x(s)*W1[65] + b1,  rel(t) = (2t-3)/4

Strategy (8 cores, shard = (batch, H-half); all device math fp16 except
fp32 PSUM accumulation and fp32 output):
  - F = W1c^T x on PE with a 128x128 block-diagonal stationary so two
    64-channel pixel groups share each streamed column.
  - F copied PSUM->SBUF as fp16 (DVE cast), then all 16 bias+relu
    variants on DVE fp16 tensor_scalar (hits the 4X perf mode).
  - pred = h @ W2 on PE with a [128,32] zero-padded block-diag
    stationary, packed 3 matmuls per PSUM tile at column-tile positions
    0/32/64; j-dim = r so each partition line holds 4 consecutive
    output rows.  Stationary columns are ordered c-major (k = 2c+g) so
    one output DMA covers both pixel groups with a [3,2,2048] AP.
  - ACT evacuates pred PSUM->SBUF; 8 DMAs per tile stream [6,2048]
    chunks (48KB) straight to DRAM, rotated across queues.
"""

import numpy as np

import concourse.bass as bass
import concourse.bacc as bacc
import concourse.mybir as mybir
import concourse.tile as tile
from concourse.bass_utils import run_bass_kernel_spmd

# Problem constants (hardcoded per contract)
B, C, H, W = 4, 64, 128, 128
OUT = 512
NF = 64
N_CORES = 8
ROWS_PER_CORE = H // 2          # 64 input rows per core
NT = 4                          # tiles of 16 input rows (8 per group)
REL = np.array([-0.75, -0.25, 0.25, 0.75], dtype=np.float32)

_CACHE = {}


def _build_program():
    if "nc" in _CACHE:
        return _CACHE["nc"]

    fp32 = mybir.dt.float32
    fp16 = mybir.dt.float16
    nc = bacc.Bacc("TRN2", target_bir_lowering=False, debug=False,
                   num_devices=N_CORES)

    # x packed on host: [part = c + 64g, t, i*128 + l] where input row
    # rho = 16t + 8g + i
    x_d = nc.dram_tensor("x", [128, NT, 8 * W], fp16, kind="ExternalInput")
    w1_d = nc.dram_tensor("w1diag", [128, 128], fp16, kind="ExternalInput")
    w2_d = nc.dram_tensor("w2diag", [128, 32], fp16, kind="ExternalInput")
    crs_d = nc.dram_tensor("crsT", [128, 16], fp32, kind="ExternalInput")
    # out row = 64t + 32g + 4i + r, laid out [c, t, g, (4i+r)*512 + l]
    out_d = nc.dram_tensor("out", [3, NT, 2, 32 * OUT], fp32,
                           kind="ExternalOutput")

    with tile.TileContext(nc) as tc:
        with (
            tc.tile_pool(name="consts", bufs=1) as consts,
            tc.tile_pool(name="xbuf", bufs=1) as xbuf,
            tc.tile_pool(name="fbuf", bufs=2) as fbuf,
            tc.tile_pool(name="hbuf", bufs=2) as hbuf,
            tc.tile_pool(name="stbuf", bufs=2) as stbuf,
            tc.tile_pool(name="fpsum", bufs=1, space=bass.MemorySpace.PSUM) as fpsum,
            tc.tile_pool(name="ppsum", bufs=3, space=bass.MemorySpace.PSUM) as ppsum,
        ):
            w1_sb = consts.tile([128, 128], fp16)
            w2_sb = consts.tile([128, 32], fp16)
            crs_sb = consts.tile([128, 16], fp32)
            xall = xbuf.tile([128, NT, 8 * W], fp16)
            nc.sync.dma_start(xall[:, 0, :], x_d[:, 0, :])
            nc.scalar.dma_start(w1_sb[:], w1_d[:])
            nc.scalar.dma_start(w2_sb[:], w2_d[:])
            nc.scalar.dma_start(crs_sb[:], crs_d[:])
            nc.sync.dma_start(xall[:, 1:NT, :], x_d[:, 1:NT, :])

            f_tiles = [None] * NT
            h_tiles = [None] * NT
            st_tiles = {}

            def feat_matmul(t, reps=1):
                ft = fpsum.tile([128, 8, W], fp32, tag="ft")
                for _ in range(reps):
                    for h in range(2):
                        nc.tensor.matmul(
                            ft[:, 4 * h:4 * h + 4, :], w1_sb[:],
                            xall[:, t, 512 * h:512 * h + 512],
                            start=True, stop=True)
                f_tiles[t] = ft

            def relus(t):
                # fb: fp16 copy of F so DVE tensor_scalar hits 4X mode
                fb = fbuf.tile([128, 8, W], fp16, tag="fb")
                nc.vector.tensor_copy(fb[:, :, :], f_tiles[t][:, :, :])
                hr = hbuf.tile([128, 16, 8, W], fp16, tag="hr")
                for v in range(16):
                    nc.vector.tensor_scalar(
                        hr[:, v, :, :], fb[:, :, :],
                        crs_sb[:, v:v + 1], 0.0,
                        mybir.AluOpType.add, mybir.AluOpType.max)
                h_tiles[t] = hr

            def dmas_for_m(t, m, engines, n0):
                # one DMA per q: [6, 2048] covering both groups and all
                # four r rows of input row i = 3q + m
                nq = 3 if m < 2 else 2
                st = st_tiles[(t, m)]
                n = n0
                for q in range(nq):
                    i = 3 * q + m
                    src = st[32 * q:32 * q + 6, :, :] \
                        .rearrange("p r l -> p (r l)")
                    dst = out_d[:, t, :, 4 * i * OUT:(4 * i + 4) * OUT]
                    engines[n % len(engines)].dma_start(dst, src)
                    n += 1
                return n

            def pred(t):
                hr = h_tiles[t]
                for m in range(3):
                    np_ = 70 if m < 2 else 38
                    stt = stbuf.tile([np_, 4, OUT], fp32,
                                     tag=f"st{m}", name=f"st{m}")
                    st_tiles[(t, m)] = stt
                ndma = 0
                # m-major: each m's output DMAs can start as soon as its
                # two pred tiles are evacuated
                for m in range(3):
                    nq = 3 if m < 2 else 2
                    np_ = 70 if m < 2 else 38
                    for rh in range(2):
                        pt = ppsum.tile([96, 2, OUT], fp32, tag="pt")
                        for q in range(nq):
                            i = 3 * q + m
                            for j in range(2):
                                r = 2 * rh + j
                                # rhs streamed s-major with l contiguous
                                # (fast PE streaming); the host gather
                                # un-permutes columns from (s,l) to 4l+s
                                rhs = hr[:, 4 * r:4 * r + 4, i, :]
                                nc.tensor.matmul(
                                    pt[32 * q:32 * q + 32, j, :],
                                    w2_sb[:], rhs, start=True, stop=True)
                        nc.scalar.activation(
                            st_tiles[(t, m)][:, 2 * rh:2 * rh + 2, :],
                            pt[0:np_, :, :],
                            mybir.ActivationFunctionType.Copy)
                    engines = ([nc.scalar, nc.sync, nc.gpsimd]
                               if t == NT - 1 else [nc.sync, nc.gpsimd])
                    ndma = dmas_for_m(t, m, engines, ndma)

            feat_matmul(0)
            for t in range(NT):
                relus(t)
                if t + 1 < NT:
                    # extra reps keep the PE HAM-warm through the relu
                    # windows (PE would otherwise micro-idle and throttle
                    # down to 1.2 GHz, slowing every subsequent matmul)
                    feat_matmul(t + 1, reps=6 if t == 0 else 3)
                pred(t)

    nc.compile()
    _CACHE["nc"] = nc
    return nc


def _prep_inputs(x, W1, b1, W2, b2):
    x = np.asarray(x, dtype=np.float32)
    W1 = np.asarray(W1, dtype=np.float32)
    b1 = np.asarray(b1, dtype=np.float32)
    W2 = np.asarray(W2, dtype=np.float32)

    w1c = W1[:NF]                      # [64, 64]
    w1diag = np.zeros((128, 128), dtype=np.float16)
    w1diag[0:64, 0:64] = w1c
    w1diag[64:128, 64:128] = w1c

    # stationary columns k = 2c + g (c-major) so the output DMA's
    # partition iteration matches a [3, 2, 2048] DRAM AP
    w2diag = np.zeros((128, 32), dtype=np.float16)
    for g in range(2):
        for ch in range(3):
            w2diag[64 * g:64 * g + 64, 2 * ch + g] = W2[:, ch]

    # c[v=4r+s, phi] = rel[r]*W1[64] + rel[s]*W1[65] + b1 -> [16, 64]
    crs = (REL[:, None, None] * W1[NF][None, None, :]
           + REL[None, :, None] * W1[NF + 1][None, None, :]
           + b1[None, None, :]).reshape(16, NF)
    crsT = np.ascontiguousarray(
        np.concatenate([crs.T, crs.T], axis=0)).astype(np.float32)  # [128,16]

    in_maps = []
    for c in range(N_CORES):
        b, half = c // 2, c % 2
        xs = x[b, :, half * ROWS_PER_CORE:(half + 1) * ROWS_PER_CORE, :]
        # [c, rho, l] -> [c, t, g, i, l] -> [g, c, t, i, l] -> [128, t, i*l]
        xp = np.ascontiguousarray(
            xs.reshape(NF, NT, 2, 8, W).transpose(2, 0, 1, 3, 4)
            .reshape(128, NT, 8 * W).astype(np.float16))
        in_maps.append({"x": xp, "w1diag": w1diag, "w2diag": w2diag,
                        "crsT": crsT})
    return in_maps


def _gather(results, b2):
    full = np.empty((B, 3, OUT, OUT), dtype=np.float32)
    for c in range(N_CORES):
        b, half = c // 2, c % 2
        # [3, t, g, 32*512] -> rows ordered (t, g, 4i+r); columns come
        # back (s, l)-ordered, un-permute to 4l+s here
        o = (results[c]["out"].reshape(3, 4 * ROWS_PER_CORE, 4, W)
             .transpose(0, 1, 3, 2).reshape(3, 4 * ROWS_PER_CORE, OUT))
        full[b, :, half * (OUT // 2):(half + 1) * (OUT // 2), :] = o
    b2 = np.asarray(b2, dtype=np.float32)
    if np.any(b2):
        full += b2.reshape(1, 3, 1, 1)
    return full


def run(trace=False, **inputs):
    nc = _build_program()
    in_maps = _prep_inputs(inputs["x"], inputs["W1"], inputs["b1"],
                           inputs["W2"], inputs["b2"])
    res = run_bass_kernel_spmd(nc, in_maps, list(range(N_CORES)), trace=trace)
    return _gather(res.results, inputs["b2"]), res


def kernel(**inputs):
    out, _ = run(trace=False, **inputs)
    return out


# revision 15
# speedup vs baseline: 2.2836x; 1.0435x over previous
"""Trainium2 Bass kernel for nn_MLP_Interpolate.

Reference computation (out_size=512, H=W=128 -> exact 4x nearest upsample):
  out[b, :, 4k+r, 4l+s] = relu(x[b,:,k,l] @ W1[:64] + c[r,s]) @ W2 + b2
  c[r,s] = rel_y(r)*W1[64] + rel_x(s)*W1[65] + b1,  rel(t) = (2t-3)/4

Strategy (8 cores, shard = (batch, H-half); all device math fp16 except
fp32 PSUM accumulation and fp32 output):
  - F = W1c^T x on PE with a 128x128 block-diagonal stationary so two
    64-channel pixel groups share each streamed column.
  - F copied PSUM->SBUF as fp16 (DVE cast), then all 16 bias+relu
    variants on DVE fp16 tensor_scalar (hits the 4X perf mode).
  - pred = h @ W2 on PE with a [128,32] zero-padded block-diag
    stationary, packed 3 matmuls per PSUM tile at column-tile positions
    0/32/64; j-dim = r so each partition line holds 4 consecutive
    output rows.  Stationary columns are ordered c-major (k = 2c+g) so
    one output DMA covers both pixel groups with a [3,2,2048] AP.
  - ACT evacuates pred PSUM->SBUF; 8 DMAs per tile stream [6,2048]
    chunks (48KB) straight to DRAM, rotated across queues.
"""

import numpy as np

import concourse.bass as bass
import concourse.bacc as bacc
import concourse.mybir as mybir
import concourse.tile as tile
from concourse.bass_utils import run_bass_kernel_spmd

# Problem constants (hardcoded per contract)
B, C, H, W = 4, 64, 128, 128
OUT = 512
NF = 64
N_CORES = 8
ROWS_PER_CORE = H // 2          # 64 input rows per core
NT = 4                          # tiles of 16 input rows (8 per group)
REL = np.array([-0.75, -0.25, 0.25, 0.75], dtype=np.float32)

_CACHE = {}


def _build_program():
    if "nc" in _CACHE:
        return _CACHE["nc"]

    fp32 = mybir.dt.float32
    fp16 = mybir.dt.float16
    nc = bacc.Bacc("TRN2", target_bir_lowering=False, debug=False,
                   num_devices=N_CORES)

    # x packed on host: [part = c + 64g, t, i*128 + l] where input row
    # rho = 16t + 8g + i
    x_d = nc.dram_tensor("x", [128, NT, 8 * W], fp16, kind="ExternalInput")
    w1_d = nc.dram_tensor("w1diag", [128, 128], fp16, kind="ExternalInput")
    w2_d = nc.dram_tensor("w2diag", [128, 32], fp16, kind="ExternalInput")
    crs_d = nc.dram_tensor("crsT", [128, 16], fp32, kind="ExternalInput")
    # out row = 64t + 32g + 4i + r, laid out [c, t, g, (4i+r)*512 + l]
    out_d = nc.dram_tensor("out", [3, NT, 2, 32 * OUT], fp32,
                           kind="ExternalOutput")

    with tile.TileContext(nc) as tc:
        with (
            tc.tile_pool(name="consts", bufs=1) as consts,
            tc.tile_pool(name="xbuf", bufs=1) as xbuf,
            tc.tile_pool(name="fbuf", bufs=2) as fbuf,
            tc.tile_pool(name="hbuf", bufs=2) as hbuf,
            tc.tile_pool(name="stbuf", bufs=2) as stbuf,
            tc.tile_pool(name="fpsum", bufs=1, space=bass.MemorySpace.PSUM) as fpsum,
            tc.tile_pool(name="ppsum", bufs=3, space=bass.MemorySpace.PSUM) as ppsum,
        ):
            w1_sb = consts.tile([128, 128], fp16)
            w2_sb = consts.tile([128, 32], fp16)
            crs_sb = consts.tile([128, 16], fp32)
            xall = xbuf.tile([128, NT, 8 * W], fp16)
            nc.sync.dma_start(xall[:, 0, :], x_d[:, 0, :])
            nc.scalar.dma_start(w1_sb[:], w1_d[:])
            nc.scalar.dma_start(w2_sb[:], w2_d[:])
            nc.scalar.dma_start(crs_sb[:], crs_d[:])
            nc.sync.dma_start(xall[:, 1:NT, :], x_d[:, 1:NT, :])

            f_tiles = [None] * NT
            h_tiles = [None] * NT
            st_tiles = {}

            def feat_matmul(t, reps=1):
                ft = fpsum.tile([128, 8, W], fp32, tag="ft")
                for _ in range(reps):
                    for h in range(2):
                        nc.tensor.matmul(
                            ft[:, 4 * h:4 * h + 4, :], w1_sb[:],
                            xall[:, t, 512 * h:512 * h + 512],
                            start=True, stop=True)
                f_tiles[t] = ft

            def relus(t):
                # fb: fp16 copy of F so DVE tensor_scalar hits 4X mode
                fb = fbuf.tile([128, 8, W], fp16, tag="fb")
                nc.vector.tensor_copy(fb[:, :, :], f_tiles[t][:, :, :])
                hr = hbuf.tile([128, 16, 8, W], fp16, tag="hr")
                for v in range(16):
                    nc.vector.tensor_scalar(
                        hr[:, v, :, :], fb[:, :, :],
                        crs_sb[:, v:v + 1], 0.0,
                        mybir.AluOpType.add, mybir.AluOpType.max)
                h_tiles[t] = hr

            def dmas_for_m(t, m, engines, n0):
                # one DMA per q: [6, 2048] covering both groups and all
                # four r rows of input row i = 3q + m
                nq = 3 if m < 2 else 2
                st = st_tiles[(t, m)]
                n = n0
                for q in range(nq):
                    i = 3 * q + m
                    src = st[32 * q:32 * q + 6, :, :] \
                        .rearrange("p r l -> p (r l)")
                    dst = out_d[:, t, :, 4 * i * OUT:(4 * i + 4) * OUT]
                    engines[n % len(engines)].dma_start(dst, src)
                    n += 1
                return n

            def pred(t):
                hr = h_tiles[t]
                for m in range(3):
                    np_ = 70 if m < 2 else 38
                    stt = stbuf.tile([np_, 4, OUT], fp32,
                                     tag=f"st{m}", name=f"st{m}")
                    st_tiles[(t, m)] = stt
                ndma = 0
                # m-major: each m's output DMAs can start as soon as its
                # two pred tiles are evacuated
                for m in range(3):
                    nq = 3 if m < 2 else 2
                    np_ = 70 if m < 2 else 38
                    for rh in range(2):
                        pt = ppsum.tile([96, 2, OUT], fp32, tag="pt")
                        for q in range(nq):
                            i = 3 * q + m
                            for j in range(2):
                                r = 2 * rh + j
                                # rhs streamed s-major with l contiguous
                                # (fast PE streaming); the host gather
                                # un-permutes columns from (s,l) to 4l+s
                                rhs = hr[:, 4 * r:4 * r + 4, i, :]
                                nc.tensor.matmul(
                                    pt[32 * q:32 * q + 32, j, :],
                                    w2_sb[:], rhs, start=True, stop=True)
                        nc.scalar.activation(
                            st_tiles[(t, m)][:, 2 * rh:2 * rh + 2, :],
                            pt[0:np_, :, :],
                            mybir.ActivationFunctionType.Copy)
                    # HWDGE queues only: gpsimd SWDGE descriptor writes
                    # go through the SBUF port shared with DVE and slow
                    # down the relu tensor_scalars
                    engines = ([nc.sync, nc.scalar]
                               if t == NT - 1 else [nc.sync])
                    ndma = dmas_for_m(t, m, engines, ndma)

            feat_matmul(0)
            for t in range(NT):
                relus(t)
                if t + 1 < NT:
                    # extra reps keep the PE HAM-warm through the relu
                    # windows (PE would otherwise micro-idle and throttle
                    # down to 1.2 GHz, slowing every subsequent matmul)
                    feat_matmul(t + 1, reps=6 if t == 0 else 3)
                pred(t)

    nc.compile()
    _CACHE["nc"] = nc
    return nc


def _prep_inputs(x, W1, b1, W2, b2):
    x = np.asarray(x, dtype=np.float32)
    W1 = np.asarray(W1, dtype=np.float32)
    b1 = np.asarray(b1, dtype=np.float32)
    W2 = np.asarray(W2, dtype=np.float32)

    w1c = W1[:NF]                      # [64, 64]
    w1diag = np.zeros((128, 128), dtype=np.float16)
    w1diag[0:64, 0:64] = w1c
    w1diag[64:128, 64:128] = w1c

    # stationary columns k = 2c + g (c-major) so the output DMA's
    # partition iteration matches a [3, 2, 2048] DRAM AP
    w2diag = np.zeros((128, 32), dtype=np.float16)
    for g in range(2):
        for ch in range(3):
            w2diag[64 * g:64 * g + 64, 2 * ch + g] = W2[:, ch]

    # c[v=4r+s, phi] = rel[r]*W1[64] + rel[s]*W1[65] + b1 -> [16, 64]
    crs = (REL[:, None, None] * W1[NF][None, None, :]
           + REL[None, :, None] * W1[NF + 1][None, None, :]
           + b1[None, None, :]).reshape(16, NF)
    crsT = np.ascontiguousarray(
        np.concatenate([crs.T, crs.T], axis=0)).astype(np.float32)  # [128,16]

    in_maps = []
    for c in range(N_CORES):
        b, half = c // 2, c % 2
        xs = x[b, :, half * ROWS_PER_CORE:(half + 1) * ROWS_PER_CORE, :]
        # [c, rho, l] -> [c, t, g, i, l] -> [g, c, t, i, l] -> [128, t, i*l]
        xp = np.ascontiguousarray(
            xs.reshape(NF, NT, 2, 8, W).transpose(2, 0, 1, 3, 4)
            .reshape(128, NT, 8 * W).astype(np.float16))
        in_maps.append({"x": xp, "w1diag": w1diag, "w2diag": w2diag,
                        "crsT": crsT})
    return in_maps


def _gather(results, b2):
    full = np.empty((B, 3, OUT, OUT), dtype=np.float32)
    for c in range(N_CORES):
        b, half = c // 2, c % 2
        # [3, t, g, 32*512] -> rows ordered (t, g, 4i+r); columns come
        # back (s, l)-ordered, un-permute to 4l+s here
        o = (results[c]["out"].reshape(3, 4 * ROWS_PER_CORE, 4, W)
             .transpose(0, 1, 3, 2).reshape(3, 4 * ROWS_PER_CORE, OUT))
        full[b, :, half * (OUT // 2):(half + 1) * (OUT // 2), :] = o
    b2 = np.asarray(b2, dtype=np.float32)
    if np.any(b2):
        full += b2.reshape(1, 3, 1, 1)
    return full


def run(trace=False, **inputs):
    nc = _build_program()
    in_maps = _prep_inputs(inputs["x"], inputs["W1"], inputs["b1"],
                           inputs["W2"], inputs["b2"])
    res = run_bass_kernel_spmd(nc, in_maps, list(range(N_CORES)), trace=trace)
    return _gather(res.results, inputs["b2"]), res


def kernel(**inputs):
    out, _ = run(trace=False, **inputs)
    return out


# revision 17
# speedup vs baseline: 2.4297x; 1.0640x over previous
"""Trainium2 Bass kernel for nn_MLP_Interpolate.

Reference computation (out_size=512, H=W=128 -> exact 4x nearest upsample):
  out[b, :, 4k+r, 4l+s] = relu(x[b,:,k,l] @ W1[:64] + c[r,s]) @ W2 + b2
  c[r,s] = rel_y(r)*W1[64] + rel_x(s)*W1[65] + b1,  rel(t) = (2t-3)/4

Strategy (8 cores, shard = (batch, H-half); all device math fp16 except
fp32 PSUM accumulation and fp32 output):
  - F = W1c^T x on PE with a 128x128 block-diagonal stationary so two
    64-channel pixel groups share each streamed column.
  - F copied PSUM->SBUF as fp16 (DVE cast), then all 16 bias+relu
    variants on DVE fp16 tensor_scalar (hits the 4X perf mode).
  - pred = h @ W2 on PE with a [128,32] zero-padded block-diag
    stationary, packed 3 matmuls per PSUM tile at column-tile positions
    0/32/64; j-dim = r so each partition line holds 4 consecutive
    output rows.  Stationary columns are ordered c-major (k = 2c+g) so
    one output DMA covers both pixel groups with a [3,2,2048] AP.
  - ACT evacuates pred PSUM->SBUF; 8 DMAs per tile stream [6,2048]
    chunks (48KB) straight to DRAM, rotated across queues.
"""

import numpy as np

import concourse.bass as bass
import concourse.bacc as bacc
import concourse.mybir as mybir
import concourse.tile as tile
from concourse.bass_utils import run_bass_kernel_spmd

# Problem constants (hardcoded per contract)
B, C, H, W = 4, 64, 128, 128
OUT = 512
NF = 64
N_CORES = 8
ROWS_PER_CORE = H // 2          # 64 input rows per core
NT = 4                          # tiles of 16 input rows (8 per group)
REL = np.array([-0.75, -0.25, 0.25, 0.75], dtype=np.float32)

_CACHE = {}


def _build_program():
    if "nc" in _CACHE:
        return _CACHE["nc"]

    fp32 = mybir.dt.float32
    fp16 = mybir.dt.float16
    nc = bacc.Bacc("TRN2", target_bir_lowering=False, debug=False,
                   num_devices=N_CORES)

    # x packed on host: [part = c + 64g, t, i*128 + l] where input row
    # rho = 16t + 8g + i
    x_d = nc.dram_tensor("x", [128, NT, 8 * W], fp16, kind="ExternalInput")
    w1_d = nc.dram_tensor("w1diag", [128, 128], fp16, kind="ExternalInput")
    w2_d = nc.dram_tensor("w2diag", [128, 32], fp16, kind="ExternalInput")
    crs_d = nc.dram_tensor("crsT", [128, 16], fp32, kind="ExternalInput")
    # out row = 64t + 32g + 4i + r, laid out [c, t, g, (4i+r)*512 + l]
    out_d = nc.dram_tensor("out", [3, NT, 2, 32 * OUT], fp32,
                           kind="ExternalOutput")

    with tile.TileContext(nc) as tc:
        with (
            tc.tile_pool(name="consts", bufs=1) as consts,
            tc.tile_pool(name="xbuf", bufs=1) as xbuf,
            tc.tile_pool(name="fbuf", bufs=2) as fbuf,
            tc.tile_pool(name="hbuf", bufs=2) as hbuf,
            tc.tile_pool(name="stbuf", bufs=2) as stbuf,
            tc.tile_pool(name="fpsum", bufs=1, space=bass.MemorySpace.PSUM) as fpsum,
            tc.tile_pool(name="ppsum", bufs=3, space=bass.MemorySpace.PSUM) as ppsum,
        ):
            w1_sb = consts.tile([128, 128], fp16)
            w2_sb = consts.tile([128, 32], fp16)
            crs_sb = consts.tile([128, 16], fp32)
            xall = xbuf.tile([128, NT, 8 * W], fp16)
            nc.sync.dma_start(xall[:, 0, :], x_d[:, 0, :])
            nc.scalar.dma_start(w1_sb[:], w1_d[:])
            nc.scalar.dma_start(w2_sb[:], w2_d[:])
            nc.scalar.dma_start(crs_sb[:], crs_d[:])
            nc.sync.dma_start(xall[:, 1:NT, :], x_d[:, 1:NT, :])

            f_tiles = [None] * NT
            h_tiles = [None] * NT
            st_tiles = {}

            def feat_matmul(t, reps=1):
                ft = fpsum.tile([128, 8, W], fp32, tag="ft")
                for _ in range(reps):
                    for h in range(2):
                        nc.tensor.matmul(
                            ft[:, 4 * h:4 * h + 4, :], w1_sb[:],
                            xall[:, t, 512 * h:512 * h + 512],
                            start=True, stop=True)
                f_tiles[t] = ft

            def relus(t):
                # fb: fp16 copy of F so DVE tensor_scalar hits 4X mode
                fb = fbuf.tile([128, 8, W], fp16, tag="fb")
                nc.vector.tensor_copy(fb[:, :, :], f_tiles[t][:, :, :])
                hr = hbuf.tile([128, 16, 8, W], fp16, tag="hr")
                # At t=0 ACT is otherwise idle (no pred copies yet), so
                # giving it the tail variants shortens the prologue.
                act_v = set(range(11, 16)) if t == 0 else ()
                for v in range(16):
                    if v in act_v:
                        nc.scalar.activation(
                            hr[:, v, :, :], fb[:, :, :],
                            mybir.ActivationFunctionType.Relu,
                            bias=crs_sb[:, v:v + 1])
                    else:
                        nc.vector.tensor_scalar(
                            hr[:, v, :, :], fb[:, :, :],
                            crs_sb[:, v:v + 1], 0.0,
                            mybir.AluOpType.add, mybir.AluOpType.max)
                h_tiles[t] = hr

            def dmas_for_m(t, m, engines, n0):
                # one DMA per q: [6, 2048] covering both groups and all
                # four r rows of input row i = 3q + m
                nq = 3 if m < 2 else 2
                st = st_tiles[(t, m)]
                n = n0
                for q in range(nq):
                    i = 3 * q + m
                    src = st[32 * q:32 * q + 6, :, :] \
                        .rearrange("p r l -> p (r l)")
                    dst = out_d[:, t, :, 4 * i * OUT:(4 * i + 4) * OUT]
                    engines[n % len(engines)].dma_start(dst, src)
                    n += 1
                return n

            def pred(t):
                hr = h_tiles[t]
                for m in range(3):
                    np_ = 70 if m < 2 else 38
                    stt = stbuf.tile([np_, 4, OUT], fp32,
                                     tag=f"st{m}", name=f"st{m}")
                    st_tiles[(t, m)] = stt
                ndma = 0
                # m-major: each m's output DMAs can start as soon as its
                # two pred tiles are evacuated
                for m in range(3):
                    nq = 3 if m < 2 else 2
                    np_ = 70 if m < 2 else 38
                    for rh in range(2):
                        pt = ppsum.tile([96, 2, OUT], fp32, tag="pt")
                        for q in range(nq):
                            i = 3 * q + m
                            for j in range(2):
                                r = 2 * rh + j
                                # rhs streamed s-major with l contiguous
                                # (fast PE streaming); the host gather
                                # un-permutes columns from (s,l) to 4l+s
                                rhs = hr[:, 4 * r:4 * r + 4, i, :]
                                nc.tensor.matmul(
                                    pt[32 * q:32 * q + 32, j, :],
                                    w2_sb[:], rhs, start=True, stop=True)
                        # m2 copies at t=3 go to DVE (idle by then) so the
                        # final DMAs can start sooner
                        if t == NT - 1 and m == 2:
                            nc.vector.tensor_copy(
                                st_tiles[(t, m)][:, 2 * rh:2 * rh + 2, :],
                                pt[0:np_, :, :])
                        else:
                            nc.scalar.activation(
                                st_tiles[(t, m)][:, 2 * rh:2 * rh + 2, :],
                                pt[0:np_, :, :],
                                mybir.ActivationFunctionType.Copy)
                    # Mid-run DMAs stay off gpsimd: its SWDGE descriptor
                    # writes share the DVE SBUF port and slow the relu
                    # tensor_scalars.  By t>=2's issue window DVE is done,
                    # so gpsimd is safe to use there.
                    if t < 2:
                        engines = [nc.sync]
                    elif t == 2:
                        engines = [nc.sync, nc.gpsimd]
                    else:
                        engines = [nc.sync, nc.scalar, nc.gpsimd]
                    ndma = dmas_for_m(t, m, engines, ndma)

            feat_matmul(0)
            for t in range(NT):
                relus(t)
                if t + 1 < NT:
                    # extra reps keep the PE HAM-warm through the relu
                    # windows (PE would otherwise micro-idle and throttle
                    # down to 1.2 GHz, slowing every subsequent matmul)
                    feat_matmul(t + 1, reps=6 if t == 0 else 3)
                pred(t)

    nc.compile()
    _CACHE["nc"] = nc
    return nc


def _prep_inputs(x, W1, b1, W2, b2):
    x = np.asarray(x, dtype=np.float32)
    W1 = np.asarray(W1, dtype=np.float32)
    b1 = np.asarray(b1, dtype=np.float32)
    W2 = np.asarray(W2, dtype=np.float32)

    w1c = W1[:NF]                      # [64, 64]
    w1diag = np.zeros((128, 128), dtype=np.float16)
    w1diag[0:64, 0:64] = w1c
    w1diag[64:128, 64:128] = w1c

    # stationary columns k = 2c + g (c-major) so the output DMA's
    # partition iteration matches a [3, 2, 2048] DRAM AP
    w2diag = np.zeros((128, 32), dtype=np.float16)
    for g in range(2):
        for ch in range(3):
            w2diag[64 * g:64 * g + 64, 2 * ch + g] = W2[:, ch]

    # c[v=4r+s, phi] = rel[r]*W1[64] + rel[s]*W1[65] + b1 -> [16, 64]
    crs = (REL[:, None, None] * W1[NF][None, None, :]
           + REL[None, :, None] * W1[NF + 1][None, None, :]
           + b1[None, None, :]).reshape(16, NF)
    crsT = np.ascontiguousarray(
        np.concatenate([crs.T, crs.T], axis=0)).astype(np.float32)  # [128,16]

    in_maps = []
    for c in range(N_CORES):
        b, half = c // 2, c % 2
        xs = x[b, :, half * ROWS_PER_CORE:(half + 1) * ROWS_PER_CORE, :]
        # [c, rho, l] -> [c, t, g, i, l] -> [g, c, t, i, l] -> [128, t, i*l]
        xp = np.ascontiguousarray(
            xs.reshape(NF, NT, 2, 8, W).transpose(2, 0, 1, 3, 4)
            .reshape(128, NT, 8 * W).astype(np.float16))
        in_maps.append({"x": xp, "w1diag": w1diag, "w2diag": w2diag,
                        "crsT": crsT})
    return in_maps


def _gather(results, b2):
    full = np.empty((B, 3, OUT, OUT), dtype=np.float32)
    for c in range(N_CORES):
        b, half = c // 2, c % 2
        # [3, t, g, 32*512] -> rows ordered (t, g, 4i+r); columns come
        # back (s, l)-ordered, un-permute to 4l+s here
        o = (results[c]["out"].reshape(3, 4 * ROWS_PER_CORE, 4, W)
             .transpose(0, 1, 3, 2).reshape(3, 4 * ROWS_PER_CORE, OUT))
        full[b, :, half * (OUT // 2):(half + 1) * (OUT // 2), :] = o
    b2 = np.asarray(b2, dtype=np.float32)
    if np.any(b2):
        full += b2.reshape(1, 3, 1, 1)
    return full


def run(trace=False, **inputs):
    nc = _build_program()
    in_maps = _prep_inputs(inputs["x"], inputs["W1"], inputs["b1"],
                           inputs["W2"], inputs["b2"])
    res = run_bass_kernel_spmd(nc, in_maps, list(range(N_CORES)), trace=trace)
    return _gather(res.results, inputs["b2"]), res


def kernel(**inputs):
    out, _ = run(trace=False, **inputs)
    return out
